# revision 10
# baseline (speedup 1.0000x reference)
"""Sparse attention (freq-biased masked softmax attention) on 8 trn2 NeuronCores.

reference:
    scores = (Q @ K^T) / sqrt(D) + log(freq)     [B,H,S,S], freq broadcast on H
    scores = where(mask == 0, -1e9, scores)
    p_attn = softmax(scores, axis=-1)
    out    = p_attn @ V
    return (out, p_attn)

Kernel math: p = exp(0.125*(QK^T + 8*ln(freq*mask))) / rowsum — no explicit
rowmax needed (scores are bounded), masked entries underflow to exactly 0
(ln(clamp(0,1e-37)) * 8 ~ -680 in the exponent). The 8*ln(f*m) bias lives in
fp16 and is INJECTED INTO PSUM by an (8*I) @ lf matmul before the QK^T
matmuls accumulate on top — so the ScalarE exp produces the masked,
freq-weighted numerator directly, and its accum_out produces the softmax
denominator. The bias tile is built once per 128-query-row tile and shared
by all 12 heads.

Sharding: core c = batch b=c//4, one quarter of the query rows, all 12 heads
(freq/mask read once per core). Transposes (K^T, Q^T, p^T) run as regular
fp16 matmuls against an identity moving operand — engages the PE clock
un-throttle (HAM), unlike transpose-mode. PV contracts p^T against V chunks.
"""

import numpy as np
from contextlib import ExitStack

B, H, S, D = 2, 12, 2048, 64
NCORES = 8
QSPLIT = NCORES // B          # 4 query-quarters per batch entry
SQ = S // QSPLIT              # 512 query rows per core

_compiled_nc = None
_patched = False


def _install_walrus_wait_patch():
    """This walrus build rejects instructions carrying >1 semaphore wait
    ("Too many sync wait commands", CoreV3GenImpl setupSyncWait). Tile
    attaches all outstanding waits to single instructions (notably the
    kernel-tail Drain). Rewrite the BIR before compile: keep the last wait
    on the instruction and emit the others as standalone single-wait
    EventSemaphore instructions just before it on the same engine —
    identical semantics on an in-order sequencer."""
    global _patched
    if _patched:
        return
    _patched = True
    import orjson
    from concourse import bass2jax, bass_utils

    uid = [0]

    def _split(bir_json):
        bir = orjson.loads(bir_json)
        changed = False
        for fn in bir.get("functions", []):
            for bb in fn.get("basicblocks", []) or fn.get("blocks", []) or []:
                insts = bb.get("instructions")
                if not insts:
                    continue
                out = []
                for inst in insts:
                    si = inst.get("sync_info")
                    waits = (si or {}).get("on_wait") or []
                    if len(waits) > 1:
                        changed = True
                        for w in waits[:-1]:
                            uid[0] += 1
                            out.append({
                                "debug": inst.get("debug"),
                                "engine": inst["engine"],
                                "ins": [],
                                "name": f"splitwait-{uid[0]}-{inst['name']}",
                                "opcode": "EventSemaphore",
                                "outs": [],
                                "sync_info": {"on_update": [], "on_wait": [w]},
                            })
                        si["on_wait"] = [waits[-1]]
                    out.append(inst)
                bb["instructions"] = out
        return orjson.dumps(bir) if changed else bir_json

    orig = bass_utils.compile_bir_kernel

    def patched(bir_json, tmpdir, neff_name="file.neff"):
        return orig(_split(bir_json), tmpdir, neff_name=neff_name)

    bass2jax.compile_bir_kernel = patched
    bass_utils.compile_bir_kernel = patched


def emit(tc, q, k, v, f, m, p_out, o_out, nH, nS, nSQ, use_accum=False, use_inject=True):
    """Per-core attention program. Shapes: q [nH,nSQ,64], k,v [nH,nS,64] f32,
    f [nSQ,nS] f32, m [nSQ,nS] i32 -> p_out [nH,nSQ,nS] f32, o_out [nH,nSQ,64] f32."""
    from concourse import masks, mybir

    dt = mybir.dt
    AF = mybir.ActivationFunctionType
    nc = tc.nc

    NT = nS // 128    # k tiles (16)
    NQ = nSQ // 128   # q subtiles per core (4)
    NH2 = nS // 2     # half a score row

    with ExitStack() as ctx:
        const_pool = ctx.enter_context(tc.tile_pool(name="const", bufs=1))
        ident_f32 = const_pool.tile([128, 128], dt.float32)
        masks.make_identity(nc, ident_f32[:])
        ident_f16 = const_pool.tile([128, 128], dt.float16)
        masks.make_identity(nc, ident_f16[:])
        ident8_f16 = const_pool.tile([128, 128], dt.float16)
        masks.make_identity(nc, ident8_f16[:])
        nc.vector.tensor_scalar_mul(ident8_f16[:], ident8_f16[:], 8.0)

        # persistent per-head operands (fp16)
        per_head = ctx.enter_context(tc.tile_pool(name="per_head", bufs=1))
        kT = [per_head.tile([64, nS], dt.float16, tag=f"kT{h}", name=f"kT{h}") for h in range(nH)]
        qT = [per_head.tile([64, nSQ], dt.float16, tag=f"qT{h}", name=f"qT{h}") for h in range(nH)]
        vf = [per_head.tile([128, NT, 64], dt.float16, tag=f"v{h}", name=f"v{h}") for h in range(nH)]

        # PSUM pools; bank budget (8 x 2KB): S_a 2 + S_b 2 + pT 2 + o_a 1 + o_b 1
        S_psp = ctx.enter_context(tc.tile_pool(name="S_ps", bufs=1, space="PSUM"))
        pT_psp = ctx.enter_context(tc.tile_pool(name="pT_ps", bufs=1, space="PSUM"))
        o_psp = ctx.enter_context(tc.tile_pool(name="o_ps", bufs=1, space="PSUM"))

        # ---- prep: K^T / Q^T via fp16 matmul-transpose, loads cast by SWDGE ----
        with tc.tile_pool(name="stage", bufs=3) as stage:
            for h in range(nH):
                ks = stage.tile([128, NT, 64], dt.float16, tag="stage")
                nc.gpsimd.dma_start(ks[:], k[h].rearrange("(t p) d -> p t d", p=128))
                for half in range(2):
                    tp = pT_psp.tile([64, NH2], dt.float32, tag="pT", name="ktp")
                    for tt in range(NT // 2):
                        t = half * (NT // 2) + tt
                        nc.tensor.matmul(tp[:, tt * 128:(tt + 1) * 128], ks[:, t],
                                         ident_f16[:], start=True, stop=True)
                    lo = half * NH2
                    if (h + half) % 2 == 0:
                        nc.vector.tensor_copy(kT[h][:, lo:lo + NH2], tp[:])
                    else:
                        nc.scalar.copy(kT[h][:, lo:lo + NH2], tp[:])
                qs_ = stage.tile([128, NQ, 64], dt.float16, tag="qstage")
                nc.gpsimd.dma_start(qs_[:], q[h].rearrange("(t p) d -> p t d", p=128))
                tq = pT_psp.tile([64, NQ * 128], dt.float32, tag="pT", name="qtp")
                for t in range(NQ):
                    nc.tensor.matmul(tq[:, t * 128:(t + 1) * 128], qs_[:, t],
                                     ident_f16[:], start=True, stop=True)
                nc.scalar.copy(qT[h][:], tq[:])
                nc.gpsimd.dma_start(vf[h][:],
                                    v[h].rearrange("(t p) d -> p t d", p=128))

        # ---- main loop ----
        io_pool = ctx.enter_context(tc.tile_pool(name="io", bufs=2))
        fm_pool = ctx.enter_context(tc.tile_pool(name="fm", bufs=2))
        work = ctx.enter_context(tc.tile_pool(name="work", bufs=2))
        pn_pool = ctx.enter_context(tc.tile_pool(name="pn", bufs=3))
        stat = ctx.enter_context(tc.tile_pool(name="stat", bufs=4))

        for qt in range(NQ):
            ft = io_pool.tile([128, nS], dt.float32, tag="ft")
            nc.gpsimd.dma_start(ft[:], f[qt * 128:(qt + 1) * 128, :])
            mt = io_pool.tile([128, nS], dt.int32, tag="mt")
            nc.gpsimd.dma_start(mt[:], m[qt * 128:(qt + 1) * 128, :])
            mf = fm_pool.tile([128, nS], dt.float32, tag="mf", bufs=1)
            nc.vector.tensor_copy(mf[:], mt[:])              # int32 -> f32 (0/1)
            fm32 = fm_pool.tile([128, nS], dt.float32, tag="fm32", bufs=1)
            nc.vector.tensor_mul(fm32[:], ft[:], mf[:])      # freq*mask
            nc.vector.tensor_scalar_max(fm32[:], fm32[:], 1e-37)
            lf = fm_pool.tile([128, nS], dt.float16, tag="lf")
            nc.scalar.activation(lf[:], fm32[:], AF.Ln)      # ln(freq*mask) fp16

            for h in range(nH):
                it = qt * nH + h
                # S halves: inject 8*ln(fm) via (8I)@lf, then QK^T on top
                S_hps = []
                for half in range(2):
                    S_ps = S_psp.tile([128, NH2], dt.float32,
                                      tag=f"S_{'ab'[half]}", name="S_ps")
                    S_hps.append(S_ps)
                    lo = half * NH2
                    for j0 in range(0, NH2, 512):
                        w = min(512, NH2 - j0)
                        sl = slice(j0, j0 + w)
                        gsl = slice(lo + j0, lo + j0 + w)
                        if use_inject:
                            nc.tensor.matmul(S_ps[:, sl], ident8_f16[:], lf[:, gsl],
                                             start=True, stop=False)
                        nc.tensor.matmul(S_ps[:, sl],
                                         qT[h][:, qt * 128:(qt + 1) * 128],
                                         kT[h][:, gsl],
                                         start=not use_inject, stop=True)
                # exp(0.125*(S + 8 ln fm)) -> p16, accum -> half row-sums
                p16 = work.tile([128, nS], dt.float16, tag="p16")
                rs = stat.tile([128, 1], dt.float32, tag="rs")
                if use_accum:
                    rs2 = stat.tile([128, 2], dt.float32, tag="rs2")
                    for half in range(2):
                        nc.scalar.activation(p16[:, half * NH2:(half + 1) * NH2],
                                             S_hps[half][:], AF.Exp, scale=0.125,
                                             accum_out=rs2[:, half:half + 1])
                    nc.vector.tensor_reduce(rs[:], rs2[:], mybir.AxisListType.X,
                                            mybir.AluOpType.add)
                else:
                    for half in range(2):
                        nc.scalar.activation(p16[:, half * NH2:(half + 1) * NH2],
                                             S_hps[half][:], AF.Exp, scale=0.125)
                    nc.vector.tensor_reduce(rs[:], p16[:], mybir.AxisListType.X,
                                            mybir.AluOpType.add)
                rc = stat.tile([128, 1], dt.float32, tag="rc")
                nc.vector.reciprocal(rc[:], rs[:])
                pn = pn_pool.tile([128, nS], dt.float32, tag="pn")
                if it % 2 == 0:
                    nc.scalar.activation(pn[:], p16[:], AF.Copy, scale=rc[:])
                else:
                    nc.vector.tensor_scalar_mul(pn[:], p16[:], rc[:])
                dma_eng = (nc.sync, nc.scalar, nc.gpsimd)[it % 3]
                dma_eng.dma_start(p_out[h, qt * 128:(qt + 1) * 128, :], pn[:])

                # p^T via fp16 matmul-transpose, two PSUM halves
                pT = work.tile([128, nS], dt.float16, tag="pT_sb")
                for half in range(2):
                    tp = pT_psp.tile([128, NH2], dt.float32, tag="pT", name="ptp")
                    for tt in range(NT // 2):
                        t = half * (NT // 2) + tt
                        nc.tensor.matmul(tp[:, tt * 128:(tt + 1) * 128],
                                         p16[:, t * 128:(t + 1) * 128],
                                         ident_f16[:], start=True, stop=True)
                    lo = half * NH2
                    if (it + half) % 2 == 0:
                        nc.vector.tensor_copy(pT[:, lo:lo + NH2], tp[:])
                    else:
                        nc.scalar.copy(pT[:, lo:lo + NH2], tp[:])

                out_ps = o_psp.tile([64, 128], dt.float32, tag="o_a", name="out_ps")
                for t in range(NT):
                    nc.tensor.matmul(out_ps[:], vf[h][:, t],
                                     pT[:, t * 128:(t + 1) * 128],
                                     start=(t == 0), stop=(t == NT - 1))
                oT = stat.tile([64, 128], dt.float32, tag="oT_sb")
                nc.vector.tensor_copy(oT[:], out_ps[:])
                o_ps = o_psp.tile([128, 64], dt.float32, tag="o_b", name="o_ps")
                nc.tensor.transpose(o_ps[:], oT[:], ident_f32[:64, :64])
                ob = stat.tile([128, 64], dt.float32, tag="ob")
                nc.vector.tensor_scalar_mul(ob[:], o_ps[:], rc[:])
                nc.sync.dma_start(o_out[h, qt * 128:(qt + 1) * 128, :], ob[:])


def _build():
    from concourse import bass, mybir, tile

    dt = mybir.dt
    nc = bass.Bass("TRN2", target_bir_lowering=False, debug=False)
    q = nc.dram_tensor("q", [H, SQ, D], dt.float32, kind="ExternalInput").ap()
    k = nc.dram_tensor("k", [H, S, D], dt.float32, kind="ExternalInput").ap()
    v = nc.dram_tensor("v", [H, S, D], dt.float32, kind="ExternalInput").ap()
    f = nc.dram_tensor("f", [SQ, S], dt.float32, kind="ExternalInput").ap()
    m = nc.dram_tensor("m", [SQ, S], dt.int32, kind="ExternalInput").ap()
    p_out = nc.dram_tensor("p", [H, SQ, S], dt.float32, kind="ExternalOutput").ap()
    o_out = nc.dram_tensor("o", [H, SQ, D], dt.float32, kind="ExternalOutput").ap()

    with tile.TileContext(nc) as tc:
        emit(tc, q, k, v, f, m, p_out, o_out, H, S, SQ)
    return nc


def _in_maps(query, key, value, freq, mask):
    maps = []
    for c in range(NCORES):
        b, qq = divmod(c, QSPLIT)
        qs = qq * SQ
        maps.append({
            "q": np.ascontiguousarray(query[b, :, qs:qs + SQ, :], dtype=np.float32),
            "k": np.ascontiguousarray(key[b], dtype=np.float32),
            "v": np.ascontiguousarray(value[b], dtype=np.float32),
            "f": np.ascontiguousarray(freq[b, 0, qs:qs + SQ, :], dtype=np.float32),
            "m": np.ascontiguousarray(mask[b, 0, qs:qs + SQ, :], dtype=np.int32),
        })
    return maps


def _run(nc, in_maps, **kw):
    from concourse import bass_utils
    return bass_utils.run_bass_kernel_spmd(nc, in_maps, core_ids=list(range(NCORES)), **kw)


def _gather(results):
    p_full = np.empty((B, H, S, S), np.float32)
    o_full = np.empty((B, H, S, D), np.float32)
    for c in range(NCORES):
        b, qq = divmod(c, QSPLIT)
        qs = qq * SQ
        p_full[b, :, qs:qs + SQ, :] = results[c]["p"]
        o_full[b, :, qs:qs + SQ, :] = results[c]["o"]
    return o_full, p_full


def _get_nc():
    global _compiled_nc
    _install_walrus_wait_patch()
    if _compiled_nc is None:
        _compiled_nc = _build()
    return _compiled_nc


def kernel(query, key, value, freq, mask):
    query, key, value = np.asarray(query), np.asarray(key), np.asarray(value)
    freq, mask = np.asarray(freq), np.asarray(mask)
    nc = _get_nc()
    res = _run(nc, _in_maps(query, key, value, freq, mask))
    o_full, p_full = _gather(res.results)
    return (o_full, p_full)


# revision 12
# speedup vs baseline: 1.1978x; 1.1978x over previous
"""Sparse attention (freq-biased masked softmax attention) on 8 trn2 NeuronCores.

reference:
    scores = (Q @ K^T) / sqrt(D) + log(freq)     [B,H,S,S], freq broadcast on H
    scores = where(mask == 0, -1e9, scores)
    p_attn = softmax(scores, axis=-1)
    out    = p_attn @ V
    return (out, p_attn)

Kernel math: p = exp(0.125*(QK^T + 8*ln(freq*mask))) / rowsum — no explicit
rowmax needed (scores are bounded), masked entries underflow to exactly 0
(ln(clamp(0,1e-37)) * 8 ~ -680 in the exponent). The 8*ln(f*m) bias lives in
fp16 and is INJECTED INTO PSUM by an (8*I) @ lf matmul before the QK^T
matmuls accumulate on top — so the ScalarE exp produces the masked,
freq-weighted numerator directly, and its accum_out produces the softmax
denominator. The bias tile is built once per 128-query-row tile and shared
by all 12 heads.

Sharding: core c = batch b=c//4, one quarter of the query rows, all 12 heads
(freq/mask read once per core). Transposes (K^T, Q^T, p^T) run as regular
fp16 matmuls against an identity moving operand — engages the PE clock
un-throttle (HAM), unlike transpose-mode. PV contracts p^T against V chunks.
"""

import numpy as np
from contextlib import ExitStack

B, H, S, D = 2, 12, 2048, 64
NCORES = 8
QSPLIT = NCORES // B          # 4 query-quarters per batch entry
SQ = S // QSPLIT              # 512 query rows per core

_compiled_nc = None
_patched = False


def _install_walrus_wait_patch():
    """This walrus build rejects instructions carrying >1 semaphore wait
    ("Too many sync wait commands", CoreV3GenImpl setupSyncWait). Tile
    attaches all outstanding waits to single instructions (notably the
    kernel-tail Drain). Rewrite the BIR before compile: keep the last wait
    on the instruction and emit the others as standalone single-wait
    EventSemaphore instructions just before it on the same engine —
    identical semantics on an in-order sequencer."""
    global _patched
    if _patched:
        return
    _patched = True
    import orjson
    from concourse import bass2jax, bass_utils

    uid = [0]

    def _split(bir_json):
        bir = orjson.loads(bir_json)
        changed = False
        for fn in bir.get("functions", []):
            for bb in fn.get("basicblocks", []) or fn.get("blocks", []) or []:
                insts = bb.get("instructions")
                if not insts:
                    continue
                out = []
                for inst in insts:
                    si = inst.get("sync_info")
                    waits = (si or {}).get("on_wait") or []
                    if len(waits) > 1:
                        changed = True
                        for w in waits[:-1]:
                            uid[0] += 1
                            out.append({
                                "debug": inst.get("debug"),
                                "engine": inst["engine"],
                                "ins": [],
                                "name": f"splitwait-{uid[0]}-{inst['name']}",
                                "opcode": "EventSemaphore",
                                "outs": [],
                                "sync_info": {"on_update": [], "on_wait": [w]},
                            })
                        si["on_wait"] = [waits[-1]]
                    out.append(inst)
                bb["instructions"] = out
        return orjson.dumps(bir) if changed else bir_json

    orig = bass_utils.compile_bir_kernel

    def patched(bir_json, tmpdir, neff_name="file.neff"):
        return orig(_split(bir_json), tmpdir, neff_name=neff_name)

    bass2jax.compile_bir_kernel = patched
    bass_utils.compile_bir_kernel = patched


def emit(tc, q, k, v, f, m, p_out, o_out, nH, nS, nSQ, use_accum=False, use_inject=False):
    """Per-core attention program. Shapes: q [nH,nSQ,64], k,v [nH,nS,64] f32,
    f [nSQ,nS] f32, m [nSQ,nS] i32 -> p_out [nH,nSQ,nS] f32, o_out [nH,nSQ,64] f32."""
    from concourse import masks, mybir

    dt = mybir.dt
    AF = mybir.ActivationFunctionType
    ALU = mybir.AluOpType
    nc = tc.nc

    NT = nS // 128    # k tiles (16)
    NQ = nSQ // 128   # q subtiles per core (4)
    NH2 = nS // 2     # half a score row

    with ExitStack() as ctx:
        const_pool = ctx.enter_context(tc.tile_pool(name="const", bufs=1))
        ident_f32 = const_pool.tile([128, 128], dt.float32)
        masks.make_identity(nc, ident_f32[:])
        ident_f16 = const_pool.tile([128, 128], dt.float16)
        masks.make_identity(nc, ident_f16[:])
        ident8_f16 = const_pool.tile([128, 128], dt.float16)
        masks.make_identity(nc, ident8_f16[:])
        nc.vector.tensor_scalar_mul(ident8_f16[:], ident8_f16[:], 8.0)

        # persistent per-head operands (fp16)
        per_head = ctx.enter_context(tc.tile_pool(name="per_head", bufs=1))
        kT = [per_head.tile([64, nS], dt.float16, tag=f"kT{h}", name=f"kT{h}") for h in range(nH)]
        qT = [per_head.tile([64, nSQ], dt.float16, tag=f"qT{h}", name=f"qT{h}") for h in range(nH)]
        vf = [per_head.tile([128, NT, 64], dt.float16, tag=f"v{h}", name=f"v{h}") for h in range(nH)]

        # PSUM pools; bank budget (8 x 2KB): S_a 2 + S_b 2 + pT 2 + o_a 1 + o_b 1
        S_psp = ctx.enter_context(tc.tile_pool(name="S_ps", bufs=1, space="PSUM"))
        pT_psp = ctx.enter_context(tc.tile_pool(name="pT_ps", bufs=1, space="PSUM"))
        o_psp = ctx.enter_context(tc.tile_pool(name="o_ps", bufs=1, space="PSUM"))

        # ---- prep: K^T / Q^T via fp16 matmul-transpose, loads cast by SWDGE ----
        with tc.tile_pool(name="stage", bufs=3) as stage:
            for h in range(nH):
                ks = stage.tile([128, NT, 64], dt.float16, tag="stage")
                nc.gpsimd.dma_start(ks[:], k[h].rearrange("(t p) d -> p t d", p=128))
                tp = pT_psp.tile([64, nS], dt.float16, tag="pT", name="ktp")
                for t in range(NT):
                    nc.tensor.transpose(tp[:, t * 128:(t + 1) * 128], ks[:, t],
                                        ident_f16[:])
                if h % 2 == 0:
                    nc.vector.tensor_copy(kT[h][:], tp[:])
                else:
                    nc.scalar.copy(kT[h][:], tp[:])
                qs_ = stage.tile([128, NQ, 64], dt.float16, tag="qstage")
                nc.gpsimd.dma_start(qs_[:], q[h].rearrange("(t p) d -> p t d", p=128))
                tq = pT_psp.tile([64, NQ * 128], dt.float16, tag="pT", name="qtp")
                for t in range(NQ):
                    nc.tensor.transpose(tq[:, t * 128:(t + 1) * 128], qs_[:, t],
                                        ident_f16[:])
                nc.scalar.copy(qT[h][:], tq[:])
                nc.gpsimd.dma_start(vf[h][:],
                                    v[h].rearrange("(t p) d -> p t d", p=128))

        # ---- main loop ----
        io_pool = ctx.enter_context(tc.tile_pool(name="io", bufs=2))
        fm_pool = ctx.enter_context(tc.tile_pool(name="fm", bufs=2))
        work = ctx.enter_context(tc.tile_pool(name="work", bufs=3))
        pn_pool = ctx.enter_context(tc.tile_pool(name="pn", bufs=3))
        stat = ctx.enter_context(tc.tile_pool(name="stat", bufs=6))

        for qt in range(NQ):
            ft = io_pool.tile([128, nS], dt.float32, tag="ft", bufs=1)
            nc.gpsimd.dma_start(ft[:], f[qt * 128:(qt + 1) * 128, :])
            mt = io_pool.tile([128, nS], dt.int32, tag="mt", bufs=1)
            nc.gpsimd.dma_start(mt[:], m[qt * 128:(qt + 1) * 128, :])
            mf = fm_pool.tile([128, nS], dt.float32, tag="mf", bufs=1)
            nc.vector.tensor_copy(mf[:], mt[:])              # int32 -> f32 (0/1)
            if use_inject:
                fm32 = fm_pool.tile([128, nS], dt.float32, tag="fm32", bufs=1)
                nc.vector.tensor_mul(fm32[:], ft[:], mf[:])  # freq*mask
                nc.vector.tensor_scalar_max(fm32[:], fm32[:], 1e-37)
                lf = fm_pool.tile([128, nS], dt.float16, tag="lf")
                nc.scalar.activation(lf[:], fm32[:], AF.Ln)  # ln(freq*mask) fp16
            else:
                lf = fm_pool.tile([128, nS], dt.float16, tag="lf")
                nc.vector.tensor_mul(lf[:], ft[:], mf[:])    # freq*mask fp16

            for h in range(nH):
                it = qt * nH + h
                # S halves: inject 8*ln(fm) via (8I)@lf, then QK^T on top
                S_hps = []
                for half in range(2):
                    S_ps = S_psp.tile([128, NH2], dt.float32,
                                      tag=f"S_{'ab'[half]}", name="S_ps")
                    S_hps.append(S_ps)
                    lo = half * NH2
                    for j0 in range(0, NH2, 512):
                        w = min(512, NH2 - j0)
                        sl = slice(j0, j0 + w)
                        gsl = slice(lo + j0, lo + j0 + w)
                        if use_inject:
                            nc.tensor.matmul(S_ps[:, sl], ident8_f16[:], lf[:, gsl],
                                             start=True, stop=False)
                        nc.tensor.matmul(S_ps[:, sl],
                                         qT[h][:, qt * 128:(qt + 1) * 128],
                                         kT[h][:, gsl],
                                         start=not use_inject, stop=True)
                # exp(0.125*(S + 8 ln fm)) -> p16, accum -> half row-sums
                e16 = work.tile([128, nS], dt.float16, tag="e16", bufs=2)
                for half in range(2):
                    nc.scalar.activation(e16[:, half * NH2:(half + 1) * NH2],
                                         S_hps[half][:], AF.Exp, scale=0.125)
                if use_inject:
                    p16 = e16
                    rs = stat.tile([128, 1], dt.float32, tag="rs")
                    nc.vector.tensor_reduce(rs[:], p16[:], mybir.AxisListType.X,
                                            mybir.AluOpType.add)
                else:
                    p16 = work.tile([128, nS], dt.float16, tag="p16")
                    rs = stat.tile([128, 1], dt.float32, tag="rs")
                    nc.vector.scalar_tensor_tensor(
                        p16[:], e16[:], 0.0, lf[:], ALU.bypass, ALU.mult,
                        accum_out=rs[:])
                rc = stat.tile([128, 1], dt.float32, tag="rc")
                nc.vector.reciprocal(rc[:], rs[:])
                pn = pn_pool.tile([128, nS], dt.float32, tag="pn")
                nc.scalar.activation(pn[:], p16[:], AF.Copy, scale=rc[:])
                dma_eng = (nc.sync, nc.scalar, nc.gpsimd)[it % 3]
                dma_eng.dma_start(p_out[h, qt * 128:(qt + 1) * 128, :], pn[:])

                # p^T via fp16 transpose-mode (fp16 PSUM -> 2x-mode copy out)
                pT_ps = pT_psp.tile([128, nS], dt.float16, tag="pT", name="ptp")
                for t in range(NT):
                    nc.tensor.transpose(pT_ps[:, t * 128:(t + 1) * 128],
                                        p16[:, t * 128:(t + 1) * 128], ident_f16[:])
                pT = work.tile([128, nS], dt.float16, tag="pT_sb", bufs=2)
                nc.vector.tensor_copy(pT[:], pT_ps[:])

                out_ps = o_psp.tile([64, 128], dt.float32, tag="o_a", name="out_ps")
                for t in range(NT):
                    nc.tensor.matmul(out_ps[:], vf[h][:, t],
                                     pT[:, t * 128:(t + 1) * 128],
                                     start=(t == 0), stop=(t == NT - 1))
                oT = stat.tile([64, 128], dt.float32, tag="oT_sb")
                nc.vector.tensor_copy(oT[:], out_ps[:])
                o_ps = o_psp.tile([128, 64], dt.float32, tag="o_b", name="o_ps")
                nc.tensor.transpose(o_ps[:], oT[:], ident_f32[:64, :64])
                ob = stat.tile([128, 64], dt.float32, tag="ob")
                nc.vector.tensor_scalar_mul(ob[:], o_ps[:], rc[:])
                nc.sync.dma_start(o_out[h, qt * 128:(qt + 1) * 128, :], ob[:])


def _build():
    from concourse import bass, mybir, tile

    dt = mybir.dt
    nc = bass.Bass("TRN2", target_bir_lowering=False, debug=False)
    q = nc.dram_tensor("q", [H, SQ, D], dt.float32, kind="ExternalInput").ap()
    k = nc.dram_tensor("k", [H, S, D], dt.float32, kind="ExternalInput").ap()
    v = nc.dram_tensor("v", [H, S, D], dt.float32, kind="ExternalInput").ap()
    f = nc.dram_tensor("f", [SQ, S], dt.float32, kind="ExternalInput").ap()
    m = nc.dram_tensor("m", [SQ, S], dt.int32, kind="ExternalInput").ap()
    p_out = nc.dram_tensor("p", [H, SQ, S], dt.float32, kind="ExternalOutput").ap()
    o_out = nc.dram_tensor("o", [H, SQ, D], dt.float32, kind="ExternalOutput").ap()

    with tile.TileContext(nc) as tc:
        emit(tc, q, k, v, f, m, p_out, o_out, H, S, SQ)
    return nc


def _in_maps(query, key, value, freq, mask):
    maps = []
    for c in range(NCORES):
        b, qq = divmod(c, QSPLIT)
        qs = qq * SQ
        maps.append({
            "q": np.ascontiguousarray(query[b, :, qs:qs + SQ, :], dtype=np.float32),
            "k": np.ascontiguousarray(key[b], dtype=np.float32),
            "v": np.ascontiguousarray(value[b], dtype=np.float32),
            "f": np.ascontiguousarray(freq[b, 0, qs:qs + SQ, :], dtype=np.float32),
            "m": np.ascontiguousarray(mask[b, 0, qs:qs + SQ, :], dtype=np.int32),
        })
    return maps


def _run(nc, in_maps, **kw):
    from concourse import bass_utils
    return bass_utils.run_bass_kernel_spmd(nc, in_maps, core_ids=list(range(NCORES)), **kw)


def _gather(results):
    p_full = np.empty((B, H, S, S), np.float32)
    o_full = np.empty((B, H, S, D), np.float32)
    for c in range(NCORES):
        b, qq = divmod(c, QSPLIT)
        qs = qq * SQ
        p_full[b, :, qs:qs + SQ, :] = results[c]["p"]
        o_full[b, :, qs:qs + SQ, :] = results[c]["o"]
    return o_full, p_full


def _get_nc():
    global _compiled_nc
    _install_walrus_wait_patch()
    if _compiled_nc is None:
        _compiled_nc = _build()
    return _compiled_nc


def kernel(query, key, value, freq, mask):
    query, key, value = np.asarray(query), np.asarray(key), np.asarray(value)
    freq, mask = np.asarray(freq), np.asarray(mask)
    nc = _get_nc()
    res = _run(nc, _in_maps(query, key, value, freq, mask))
    o_full, p_full = _gather(res.results)
    return (o_full, p_full)
